# revision 24
# baseline (speedup 1.0000x reference)
"""Trainium2 Bass kernel for nn_ChannelSpatialAttention2 (dense_cnn).

Data-parallel over batch: 16 samples / 8 cores = 2 samples per core, no
cross-core communication.  Per-sample dataflow (channel-major layout
(128 ch, 16384 px), px = h*128 + w):

  1. Inputs are pre-cast to bf16 on the host, halving HBM read traffic.
     Pooled channel sums via tensor_tensor_reduce on quarter pairs.
  2. Tiny MLP (1x1 convs + BN folded on host) -> channel weights a.
  3. conv1 folded:  g_pre = (Wv + Wi*diag(a)) @ f_vi + (Wi + Wv*diag(a)) @ f_ir
     streamed per quarter; BN+ReLU fused into PSUM->SBUF activation;
     avg-pool rides accum_out.
  4. Per quarter: DMA-xbar transpose g -> gT; channel-mean map via
     segmented reduce_sum on gT, channel-max via halving tree on gT,
     pixel-max via in-place tree on g (which is dead after transpose).
  5. 7x7 spatial conv = 14 accumulating matmuls against host-built banded
     matrices (map slices stationary, dy via free-dim shifts).
  6. Blend: sa row broadcast across partitions (gpsimd), tt =
     sigmoid(ca*sa) on ACT with per-partition scale; dt2 = (1-a)*tt*(fv-fi)
     on DVE; out = diag(a)@fv + fi (+ dt2) accumulated in PSUM with the
     last term either as a third matmul + ACT evac or added during a DVE
     evac (split tunable to balance engines).  Output stored bf16,
     upcast on host.
"""

import sys

if '/opt/trn_rl_repo' not in sys.path:
    sys.path.insert(0, '/opt/trn_rl_repo')

import numpy as np
import ml_dtypes

import concourse.bacc as bacc
import concourse.mybir as mybir
import concourse.tile as tile
import concourse.bass_utils as bass_utils

EPS = 1e-5
C = 128
N, H, W = 16, 128, 128
P = H * W            # 16384 pixels per sample
QP = P // 4          # 4096-px quarter
NCORES = 8
import os
SPC = int(os.environ.get('KSPC', N // NCORES))
# number of blend blocks (of 16 per sample) whose last term goes through a
# third matmul + ACT evac instead of a DVE evac-add
ACT_EVAC = int(os.environ.get('KACT_EVAC', 8))

BF16 = mybir.dt.bfloat16
F32 = mybir.dt.float32
AL = mybir.AluOpType
AF = mybir.ActivationFunctionType
AX = mybir.AxisListType

_cache = {}


def _build_program():
    nc = bacc.Bacc("TRN2", target_bir_lowering=False, debug=False,
                   enable_asserts=False, num_devices=NCORES)

    d_fvi = nc.dram_tensor("f_vi", (SPC, C, P), BF16, kind="ExternalInput").ap()
    d_fir = nc.dram_tensor("f_ir", (SPC, C, P), BF16, kind="ExternalInput").ap()
    d_out = nc.dram_tensor("out", (SPC, C, P), BF16, kind="ExternalOutput").ap()

    d_wvi = nc.dram_tensor("wvi_t", (C, C), BF16, kind="ExternalInput").ap()
    d_wir = nc.dram_tensor("wir_t", (C, C), BF16, kind="ExternalInput").ap()
    d_eye = nc.dram_tensor("eye", (C, C), BF16, kind="ExternalInput").ap()
    d_bmat = nc.dram_tensor("bmat", (14, 128, 128), BF16, kind="ExternalInput").ap()
    d_l1v = nc.dram_tensor("l1v", (C, C // 2), F32, kind="ExternalInput").ap()
    d_l1i = nc.dram_tensor("l1i", (C, C // 2), F32, kind="ExternalInput").ap()
    d_b1 = nc.dram_tensor("b1", (C // 2, 1), F32, kind="ExternalInput").ap()
    d_l2 = nc.dram_tensor("l2", (C // 2, C), F32, kind="ExternalInput").ap()
    d_b2 = nc.dram_tensor("b2", (C, 1), F32, kind="ExternalInput").ap()
    d_sc = nc.dram_tensor("sc", (C, 1), F32, kind="ExternalInput").ap()
    d_bc = nc.dram_tensor("bc", (C, 1), F32, kind="ExternalInput").ap()
    d_c1a = nc.dram_tensor("c1a", (C, 8), F32, kind="ExternalInput").ap()
    d_c1m = nc.dram_tensor("c1m", (C, 8), F32, kind="ExternalInput").ap()
    d_c2r = nc.dram_tensor("c2r", (8, C), F32, kind="ExternalInput").ap()

    with tile.TileContext(nc) as tc:
        with (
            tc.tile_pool(name="wts", bufs=1) as wts,
            tc.tile_pool(name="io", bufs=6) as io,
            tc.tile_pool(name="gq", bufs=2) as gqp,
            tc.tile_pool(name="gtb", bufs=2) as gtb,
            tc.tile_pool(name="scr", bufs=1) as scr,
            tc.tile_pool(name="sm", bufs=2) as sm,
            tc.tile_pool(name="mp", bufs=2) as mp,
            tc.tile_pool(name="bl", bufs=2) as bl,
            tc.tile_pool(name="ob", bufs=3) as obp,
            tc.tile_pool(name="sfl", bufs=2) as sfl,
            tc.tile_pool(name="ps", bufs=4, space="PSUM") as ps,
        ):
            # ---- constant loads ----
            wvi = wts.tile([C, C], BF16)
            wir = wts.tile([C, C], BF16)
            eye = wts.tile([C, C], BF16)
            bmat = wts.tile([128, 14, 128], BF16)
            nc.gpsimd.dma_start(wvi[:], d_wvi[:])
            nc.gpsimd.dma_start(wir[:], d_wir[:])
            nc.gpsimd.dma_start(eye[:], d_eye[:])
            nc.gpsimd.dma_start(bmat[:], d_bmat.rearrange("m r c -> r m c"))
            l1v = wts.tile([C, C // 2], F32)
            l1i = wts.tile([C, C // 2], F32)
            b1 = wts.tile([C // 2, 1], F32)
            l2 = wts.tile([C // 2, C], F32)
            b2 = wts.tile([C, 1], F32)
            sc = wts.tile([C, 1], F32)
            bc = wts.tile([C, 1], F32)
            c1a = wts.tile([C, 8], F32)
            c1m = wts.tile([C, 8], F32)
            c2r = wts.tile([8, C], F32)
            for t, d in ((l1v, d_l1v), (l1i, d_l1i), (b1, d_b1), (l2, d_l2),
                         (b2, d_b2), (sc, d_sc), (bc, d_bc), (c1a, d_c1a),
                         (c1m, d_c1m), (c2r, d_c2r)):
                nc.gpsimd.dma_start(t[:], d[:])

            for s in range(SPC):
                # ---- loads (bf16 in HBM, no cast) ----
                fvq = [io.tile([C, QP], BF16, tag="fv", name=f"fv{s}_{k}")
                       for k in range(4)]
                fiq = [io.tile([C, QP], BF16, tag="fi", name=f"fi{s}_{k}")
                      for k in range(4)]
                for k in range(4):
                    sl = slice(k * QP, (k + 1) * QP)
                    nc.gpsimd.dma_start(fvq[k][:], d_fvi[s][:, sl])
                    nc.gpsimd.dma_start(fiq[k][:], d_fir[s][:, sl])

                def FV(sl):
                    k = sl.start // QP
                    return fvq[k][:, sl.start - k * QP:sl.stop - k * QP]

                def FI(sl):
                    k = sl.start // QP
                    return fiq[k][:, sl.start - k * QP:sl.stop - k * QP]

                # ---- pooled channel sums: quarters folded to (C, 2048) by
                # DMA-accumulate (CCE inline add), short DVE tree for the rest
                sv = sm.tile([C, 1], F32, tag="sv", name=f"sv{s}")
                si = sm.tile([C, 1], F32, tag="si", name=f"si{s}")
                for qt, out_col, nm in ((fvq, sv, "av"), (fiq, si, "ai")):
                    acc = scr.tile([C, 2048], BF16, tag=f"acc{nm}", name=f"acc{nm}{s}")
                    nc.gpsimd.dma_start(acc[:], qt[0][:, 0:2048])
                    nc.gpsimd.dma_start(acc[:], qt[0][:, 2048:QP], accum_op=AL.add)
                    for k in range(1, 4):
                        nc.gpsimd.dma_start(acc[:], qt[k][:, 0:2048], accum_op=AL.add)
                        nc.gpsimd.dma_start(acc[:], qt[k][:, 2048:QP], accum_op=AL.add)
                    nc.vector.tensor_tensor(out=acc[:, 0:1024], in0=acc[:, 0:1024],
                                            in1=acc[:, 1024:2048], op=AL.add)
                    nc.vector.tensor_tensor(out=acc[:, 0:512], in0=acc[:, 0:512],
                                            in1=acc[:, 512:1024], op=AL.add)
                    nc.vector.reduce_sum(out_col[:], acc[:, 0:512], axis=AX.X)

                # ---- channel-avg-attention MLP -> a ----
                ps1 = ps.tile([C, 1024], F32, tag="ps", name=f"ps1_{s}")
                nc.tensor.matmul(ps1[0:64, 0:1], l1v[:], sv[:], start=True, stop=False)
                nc.tensor.matmul(ps1[0:64, 0:1], l1i[:], si[:], start=False, stop=True)
                h1 = sm.tile([C // 2, 1], F32, tag="h1", name=f"h1_{s}")
                nc.scalar.activation(h1[:], ps1[0:64, 0:1], AF.Relu, bias=b1[:])
                ps2 = ps.tile([C, 1024], F32, tag="ps", name=f"ps2_{s}")
                nc.tensor.matmul(ps2[0:C, 0:1], l2[:], h1[:], start=True, stop=True)
                a_col = sm.tile([C, 1], F32, tag="a_col", name=f"a_col{s}")
                nc.scalar.activation(a_col[:], ps2[0:C, 0:1], AF.Sigmoid, bias=b2[:])
                oma = sm.tile([C, 1], F32, tag="oma", name=f"oma{s}")
                nc.vector.tensor_scalar(oma[:], a_col[:], -1.0, 1.0, AL.mult, AL.add)

                # per-sample effective conv1 weights + diag(a), diag(1-a)
                lv = sm.tile([C, C], BF16, tag="lv", name=f"lv{s}")
                li = sm.tile([C, C], BF16, tag="li", name=f"li{s}")
                dga = sm.tile([C, C], BF16, tag="dga", name=f"dga{s}")
                dgo = sm.tile([C, C], BF16, tag="dgo", name=f"dgo{s}")
                nc.vector.scalar_tensor_tensor(lv[:], wir[:], a_col[:], wvi[:], AL.mult, AL.add)
                nc.vector.scalar_tensor_tensor(li[:], wvi[:], a_col[:], wir[:], AL.mult, AL.add)
                nc.vector.tensor_scalar(dga[:], eye[:], a_col[:], 0.0, AL.mult, AL.add)
                nc.vector.tensor_scalar(dgo[:], eye[:], oma[:], 0.0, AL.mult, AL.add)

                # ---- conv1 + per-quarter maps ----
                avp = sm.tile([C, 16], F32, tag="avp", name=f"avp{s}")
                rmax = sm.tile([C, 128], BF16, tag="rmax", name=f"rmax{s}")
                maxpad = mp.tile([128, 134], BF16, tag="maxpad", name=f"maxpad{s}")
                sumpad = mp.tile([128, 134], BF16, tag="sumpad", name=f"sumpad{s}")
                nc.vector.memset(maxpad[:, 0:3], 0.0)
                nc.vector.memset(maxpad[:, 131:134], 0.0)
                nc.vector.memset(sumpad[:, 0:3], 0.0)
                nc.vector.memset(sumpad[:, 131:134], 0.0)

                for k in range(4):
                    gq = gqp.tile([C, QP], BF16, tag="gq", name=f"gq{s}_{k}")
                    for j in range(4):
                        pg = ps.tile([C, 1024], F32, tag="ps", name=f"pg{s}_{k}_{j}")
                        base = k * QP + j * 1024
                        # group matmuls by stationary operand (lv then li)
                        nc.tensor.matmul(pg[:, 0:512], lv[:], FV(slice(base, base + 512)),
                                         start=True, stop=False)
                        nc.tensor.matmul(pg[:, 512:1024], lv[:], FV(slice(base + 512, base + 1024)),
                                         start=True, stop=False)
                        nc.tensor.matmul(pg[:, 0:512], li[:], FI(slice(base, base + 512)),
                                         start=False, stop=True)
                        nc.tensor.matmul(pg[:, 512:1024], li[:], FI(slice(base + 512, base + 1024)),
                                         start=False, stop=True)
                        nc.scalar.activation(gq[:, j * 1024:(j + 1) * 1024], pg[:],
                                             AF.Relu, bias=bc[:], scale=sc[:],
                                             accum_out=avp[:, 4 * k + j:4 * k + j + 1])
                    # transpose quarter -> (w, h, c)
                    gt = gtb.tile([128, QP], BF16, tag="gt", name=f"gt{s}_{k}")
                    gt3 = gt[:].rearrange("p (h c) -> p h c", c=128)
                    nc.sync.dma_start_transpose(gt3, gq[:])
                    # channel-mean map: fold c 128->32 with two DMA-accumulate
                    # levels, then a short segmented DVE reduce
                    macc = scr.tile([128, 32, 64], BF16, tag="macc", name=f"macc{s}_{k}")
                    nc.gpsimd.dma_start(macc[:], gt3[:, :, 0:64])
                    nc.gpsimd.dma_start(macc[:], gt3[:, :, 64:128], accum_op=AL.add)
                    nc.gpsimd.dma_start(macc[:, :, 0:32], macc[:, :, 32:64], accum_op=AL.add)
                    msum = sm.tile([128, 32], F32, tag="msum", name=f"msum{s}_{k}")
                    nc.vector.reduce_sum(
                        msum[:].rearrange("p (f o) -> p f o", o=1),
                        macc[:, :, 0:32], axis=AX.X)
                    nc.vector.tensor_copy(
                        sumpad[:, 3 + k * 32:3 + k * 32 + 32], msum[:])
                    # channel-max tree (in place on gt)
                    w_ = 64
                    while w_ >= 1:
                        nc.vector.tensor_tensor(
                            out=gt3[:, :, 0:w_], in0=gt3[:, :, 0:w_],
                            in1=gt3[:, :, w_:2 * w_], op=AL.max)
                        w_ //= 2
                    nc.vector.tensor_copy(
                        maxpad[:, 3 + k * 32:3 + k * 32 + 32].rearrange("p (f o) -> p f o", o=1),
                        gt3[:, :, 0:1])
                    # pixel-max tree in place on gq (dead after the transpose)
                    w_ = 2048
                    while w_ >= 128:
                        nc.vector.tensor_tensor(out=gq[:, 0:w_], in0=gq[:, 0:w_],
                                                in1=gq[:, w_:2 * w_], op=AL.max)
                        w_ //= 2
                    if k == 0:
                        nc.vector.tensor_copy(rmax[:], gq[:, 0:128])
                    else:
                        nc.vector.tensor_tensor(out=rmax[:], in0=rmax[:],
                                                in1=gq[:, 0:128], op=AL.max)

                avs = sm.tile([C, 1], F32, tag="avs", name=f"avs{s}")
                nc.vector.reduce_sum(avs[:], avp[:], axis=AX.X)
                mx = sm.tile([C, 1], F32, tag="mx", name=f"mx{s}")
                nc.vector.reduce_max(mx[:], rmax[:], axis=AX.X)

                # ---- ChannelAttention MLP -> ca column ----
                psa = ps.tile([C, 1024], F32, tag="ps", name=f"psa{s}")
                nc.tensor.matmul(psa[0:8, 0:1], c1a[:], avs[:], start=True, stop=True)
                ha = sm.tile([8, 1], F32, tag="ha", name=f"ha{s}")
                nc.scalar.activation(ha[:], psa[0:8, 0:1], AF.Relu)
                psm = ps.tile([C, 1024], F32, tag="ps", name=f"psm{s}")
                nc.tensor.matmul(psm[0:8, 0:1], c1m[:], mx[:], start=True, stop=True)
                hm = sm.tile([8, 1], F32, tag="hm", name=f"hm{s}")
                nc.scalar.activation(hm[:], psm[0:8, 0:1], AF.Relu)
                psr = ps.tile([C, 1024], F32, tag="ps", name=f"psr{s}")
                nc.tensor.matmul(psr[0:1, 0:C], ha[:], c2r[:], start=True, stop=False)
                nc.tensor.matmul(psr[0:1, 0:C], hm[:], c2r[:], start=False, stop=True)
                ca = sm.tile([1, C], BF16, tag="ca", name=f"ca{s}")
                nc.scalar.activation(ca[:], psr[0:1, 0:C], AF.Sigmoid)

                # ---- SpatialAttention: 7x7 conv as banded matmuls ----
                pss = ps.tile([C, 1024], F32, tag="ps", name=f"pss{s}")
                first = True
                for chn, pad in ((0, sumpad), (1, maxpad)):
                    for dy in range(7):
                        nc.tensor.matmul(pss[0:128, 0:128], pad[:, dy:dy + 128],
                                         bmat[:, chn * 7 + dy, :],
                                         start=first, stop=(chn == 1 and dy == 6))
                        first = False
                sa_hw = sm.tile([128, 128], BF16, tag="sa_hw", name=f"sa_hw{s}")
                nc.scalar.activation(sa_hw[:], pss[0:128, 0:128], AF.Sigmoid)


                # ---- blend: 8 blocks of 2048 px ----
                for b in range(8):
                    b0 = b * 2048
                    # stage block's 16 sa rows on partition 0; tt = sigmoid(ca x sa)
                    # via K=1 matmuls (rank-1 broadcast on the PE)
                    sa1p = sfl.tile([1, 2048], BF16, tag="sa1p", name=f"sa1p{s}_{b}")
                    nc.scalar.dma_start(sa1p[:], sa_hw[16 * b:16 * b + 16, :])
                    tt = bl.tile([C, 2048], BF16, tag="tt", name=f"tt{s}_{b}")
                    for j in range(2):
                        ppw = ps.tile([C, 1024], F32, tag="ps", name=f"ppw{s}_{b}_{j}")
                        nc.tensor.matmul(ppw[:, 0:512], ca[:],
                                         sa1p[0:1, j * 1024:j * 1024 + 512],
                                         start=True, stop=True)
                        nc.tensor.matmul(ppw[:, 512:1024], ca[:],
                                         sa1p[0:1, j * 1024 + 512:(j + 1) * 1024],
                                         start=True, stop=True)
                        nc.scalar.activation(tt[:, j * 1024:(j + 1) * 1024], ppw[:],
                                             AF.Sigmoid)
                    eb = bl.tile([C, 2048], BF16, tag="eb", name=f"eb{s}_{b}")
                    nc.vector.tensor_tensor(out=eb[:], in0=FV(slice(b0, b0 + 2048)),
                                            in1=FI(slice(b0, b0 + 2048)), op=AL.subtract)
                    dtb = bl.tile([C, 2048], BF16, tag="dtb", name=f"dtb{s}_{b}")
                    nc.vector.tensor_tensor(out=dtb[:], in0=tt[:], in1=eb[:], op=AL.mult)
                    for j in range(2):
                        base = b0 + j * 1024
                        act_mode = (2 * b + j) % (16 // max(1, ACT_EVAC)) == 0 if ACT_EVAC else False
                        pb = ps.tile([C, 1024], F32, tag="ps", name=f"pb{s}_{b}_{j}")
                        sla = slice(base, base + 512)
                        slb = slice(base + 512, base + 1024)
                        nc.tensor.matmul(pb[:, 0:512], dga[:], FV(sla), start=True, stop=False)
                        nc.tensor.matmul(pb[:, 512:1024], dga[:], FV(slb), start=True, stop=False)
                        if act_mode:
                            nc.tensor.matmul(pb[:, 0:512], eye[:], FI(sla), start=False, stop=False)
                            nc.tensor.matmul(pb[:, 512:1024], eye[:], FI(slb), start=False, stop=False)
                            nc.tensor.matmul(pb[:, 0:512], dgo[:],
                                             dtb[:, j * 1024:j * 1024 + 512],
                                             start=False, stop=True)
                            nc.tensor.matmul(pb[:, 512:1024], dgo[:],
                                             dtb[:, j * 1024 + 512:(j + 1) * 1024],
                                             start=False, stop=True)
                        else:
                            nc.tensor.matmul(pb[:, 0:512], eye[:], FI(sla), start=False, stop=True)
                            nc.tensor.matmul(pb[:, 512:1024], eye[:], FI(slb), start=False, stop=True)
                        ob = obp.tile([C, 1024], BF16, tag="ob", name=f"ob{s}_{b}_{j}")
                        if act_mode:
                            nc.scalar.activation(ob[:], pb[:], AF.Copy)
                        else:
                            # out = pb + (1-a)*dt, fused into the PSUM evac
                            nc.vector.scalar_tensor_tensor(
                                ob[:], dtb[:, j * 1024:(j + 1) * 1024], oma[:], pb[:],
                                AL.mult, AL.add)
                        nc.sync.dma_start(d_out[s][:, base:base + 1024], ob[:])

    nc.compile()
    return nc


def _host_consts(ca1_w, ca1_b, bn_a_g, bn_a_b, bn_a_m, bn_a_v,
                 ca2_w, ca2_b, bn_b_g, bn_b_b, bn_b_m, bn_b_v,
                 conv1_w, conv1_b, bn_c_g, bn_c_b, bn_c_m, bn_c_v,
                 chatt_w1, chatt_w2, sa_w):
    bf = ml_dtypes.bfloat16
    f = np.float32
    k_a = bn_a_g / np.sqrt(bn_a_v + EPS)
    w1 = ca1_w * k_a[:, None]
    b1 = (ca1_b - bn_a_m) * k_a + bn_a_b
    k_b = bn_b_g / np.sqrt(bn_b_v + EPS)
    w2 = ca2_w * k_b[:, None]
    b2 = (ca2_b - bn_b_m) * k_b + bn_b_b
    s_c = bn_c_g / np.sqrt(bn_c_v + EPS)
    b_c = (conv1_b - bn_c_m) * s_c + bn_c_b
    bmat = np.zeros((14, 128, 128), np.float32)
    for chn in range(2):
        scale = (1.0 / 128.0) if chn == 0 else 1.0
        for dy in range(7):
            for dx in range(7):
                off = dx - 3          # w' - w
                v = sa_w[0, chn, dy, dx] * scale
                if off >= 0:
                    idx = np.arange(0, 128 - off)
                    bmat[chn * 7 + dy, idx + off, idx] = v
                else:
                    idx = np.arange(-off, 128)
                    bmat[chn * 7 + dy, idx + off, idx] = v
    return {
        "wvi_t": np.ascontiguousarray(conv1_w[:, :C].T).astype(bf),
        "wir_t": np.ascontiguousarray(conv1_w[:, C:].T).astype(bf),
        "eye": np.eye(C, dtype=f).astype(bf),
        "bmat": bmat.astype(bf),
        "l1v": np.ascontiguousarray((w1[:, :C] / P).T).astype(f),
        "l1i": np.ascontiguousarray((w1[:, C:] / P).T).astype(f),
        "b1": b1.reshape(-1, 1).astype(f),
        "l2": np.ascontiguousarray(w2.T).astype(f),
        "b2": b2.reshape(-1, 1).astype(f),
        "sc": s_c.reshape(-1, 1).astype(f),
        "bc": b_c.reshape(-1, 1).astype(f),
        "c1a": np.ascontiguousarray((chatt_w1 / P).T).astype(f),
        "c1m": np.ascontiguousarray(chatt_w1.T).astype(f),
        "c2r": np.ascontiguousarray(chatt_w2.T).astype(f),
    }


def kernel(f_vi, f_ir, ca1_w, ca1_b, bn_a_g, bn_a_b, bn_a_m, bn_a_v,
           ca2_w, ca2_b, bn_b_g, bn_b_b, bn_b_m, bn_b_v,
           conv1_w, conv1_b, bn_c_g, bn_c_b, bn_c_m, bn_c_v,
           chatt_w1, chatt_w2, sa_w, _trace=False):
    if "nc" not in _cache:
        _cache["nc"] = _build_program()
    nc = _cache["nc"]

    consts = _host_consts(
        np.asarray(ca1_w, np.float32), np.asarray(ca1_b, np.float32),
        np.asarray(bn_a_g, np.float32), np.asarray(bn_a_b, np.float32),
        np.asarray(bn_a_m, np.float32), np.asarray(bn_a_v, np.float32),
        np.asarray(ca2_w, np.float32), np.asarray(ca2_b, np.float32),
        np.asarray(bn_b_g, np.float32), np.asarray(bn_b_b, np.float32),
        np.asarray(bn_b_m, np.float32), np.asarray(bn_b_v, np.float32),
        np.asarray(conv1_w, np.float32), np.asarray(conv1_b, np.float32),
        np.asarray(bn_c_g, np.float32), np.asarray(bn_c_b, np.float32),
        np.asarray(bn_c_m, np.float32), np.asarray(bn_c_v, np.float32),
        np.asarray(chatt_w1, np.float32), np.asarray(chatt_w2, np.float32),
        np.asarray(sa_w, np.float32))

    bf = ml_dtypes.bfloat16
    fv = np.asarray(f_vi, np.float32).reshape(N, C, P).astype(bf)
    fi = np.asarray(f_ir, np.float32).reshape(N, C, P).astype(bf)
    in_maps = []
    for i in range(NCORES):
        m = dict(consts)
        m["f_vi"] = np.ascontiguousarray(fv[i * SPC:(i + 1) * SPC])
        m["f_ir"] = np.ascontiguousarray(fi[i * SPC:(i + 1) * SPC])
        in_maps.append(m)

    res = bass_utils.run_bass_kernel_spmd(nc, in_maps, core_ids=list(range(NCORES)),
                                          trace=_trace)
    if _trace:
        _cache["last_trace"] = res
    out = np.concatenate([res.results[i]["out"] for i in range(NCORES)], axis=0)
    return out.astype(np.float32).reshape(N, C, H, W)


# revision 26
# speedup vs baseline: 2.0459x; 2.0459x over previous
"""Trainium2 Bass kernel for nn_ChannelSpatialAttention2 (dense_cnn).

Data-parallel over batch: 16 samples / 8 cores = 2 samples per core, no
cross-core communication.  Per-sample dataflow (channel-major layout
(128 ch, 16384 px), px = h*128 + w):

  1. Inputs are pre-cast to bf16 on the host, halving HBM read traffic.
     Pooled channel sums via tensor_tensor_reduce on quarter pairs.
  2. Tiny MLP (1x1 convs + BN folded on host) -> channel weights a.
  3. conv1 folded:  g_pre = (Wv + Wi*diag(a)) @ f_vi + (Wi + Wv*diag(a)) @ f_ir
     streamed per quarter; BN+ReLU fused into PSUM->SBUF activation;
     avg-pool rides accum_out.
  4. Per quarter: DMA-xbar transpose g -> gT; channel-mean map via
     segmented reduce_sum on gT, channel-max via halving tree on gT,
     pixel-max via in-place tree on g (which is dead after transpose).
  5. 7x7 spatial conv = 14 accumulating matmuls against host-built banded
     matrices (map slices stationary, dy via free-dim shifts).
  6. Blend: sa row broadcast across partitions (gpsimd), tt =
     sigmoid(ca*sa) on ACT with per-partition scale; dt2 = (1-a)*tt*(fv-fi)
     on DVE; out = diag(a)@fv + fi (+ dt2) accumulated in PSUM with the
     last term either as a third matmul + ACT evac or added during a DVE
     evac (split tunable to balance engines).  Output stored bf16,
     upcast on host.
"""

import sys

if '/opt/trn_rl_repo' not in sys.path:
    sys.path.insert(0, '/opt/trn_rl_repo')

import numpy as np
import ml_dtypes

import concourse.bacc as bacc
import concourse.mybir as mybir
import concourse.tile as tile
import concourse.bass_utils as bass_utils

EPS = 1e-5
C = 128
N, H, W = 16, 128, 128
P = H * W            # 16384 pixels per sample
QP = P // 4          # 4096-px quarter
NCORES = 8
import os
SPC = int(os.environ.get('KSPC', N // NCORES))
# number of blend blocks (of 16 per sample) whose last term goes through a
# third matmul + ACT evac instead of a DVE evac-add
ACT_EVAC = int(os.environ.get('KACT_EVAC', 8))

BF16 = mybir.dt.bfloat16
F32 = mybir.dt.float32
AL = mybir.AluOpType
AF = mybir.ActivationFunctionType
AX = mybir.AxisListType

_cache = {}


def _build_program():
    nc = bacc.Bacc("TRN2", target_bir_lowering=False, debug=False,
                   enable_asserts=False, num_devices=NCORES)

    d_fvi = nc.dram_tensor("f_vi", (SPC, C, P), BF16, kind="ExternalInput").ap()
    d_fir = nc.dram_tensor("f_ir", (SPC, C, P), BF16, kind="ExternalInput").ap()
    d_out = nc.dram_tensor("out", (SPC, C, P), BF16, kind="ExternalOutput").ap()

    d_wvi = nc.dram_tensor("wvi_t", (C, C), BF16, kind="ExternalInput").ap()
    d_wir = nc.dram_tensor("wir_t", (C, C), BF16, kind="ExternalInput").ap()
    d_eye = nc.dram_tensor("eye", (C, C), BF16, kind="ExternalInput").ap()
    d_bmat = nc.dram_tensor("bmat", (14, 128, 128), BF16, kind="ExternalInput").ap()
    d_l1v = nc.dram_tensor("l1v", (C, C // 2), F32, kind="ExternalInput").ap()
    d_l1i = nc.dram_tensor("l1i", (C, C // 2), F32, kind="ExternalInput").ap()
    d_b1 = nc.dram_tensor("b1", (C // 2, 1), F32, kind="ExternalInput").ap()
    d_l2 = nc.dram_tensor("l2", (C // 2, C), F32, kind="ExternalInput").ap()
    d_b2 = nc.dram_tensor("b2", (C, 1), F32, kind="ExternalInput").ap()
    d_sc = nc.dram_tensor("sc", (C, 1), F32, kind="ExternalInput").ap()
    d_bc = nc.dram_tensor("bc", (C, 1), F32, kind="ExternalInput").ap()
    d_c1a = nc.dram_tensor("c1a", (C, 8), F32, kind="ExternalInput").ap()
    d_c1m = nc.dram_tensor("c1m", (C, 8), F32, kind="ExternalInput").ap()
    d_c2r = nc.dram_tensor("c2r", (8, C), F32, kind="ExternalInput").ap()

    with tile.TileContext(nc) as tc:
        with (
            tc.tile_pool(name="wts", bufs=1) as wts,
            tc.tile_pool(name="io", bufs=6) as io,
            tc.tile_pool(name="gq", bufs=2) as gqp,
            tc.tile_pool(name="gtb", bufs=2) as gtb,
            tc.tile_pool(name="scr", bufs=1) as scr,
            tc.tile_pool(name="sm", bufs=2) as sm,
            tc.tile_pool(name="mp", bufs=2) as mp,
            tc.tile_pool(name="bl", bufs=2) as bl,
            tc.tile_pool(name="ob", bufs=3) as obp,
            tc.tile_pool(name="sfl", bufs=2) as sfl,
            tc.tile_pool(name="ps", bufs=4, space="PSUM") as ps,
        ):
            # ---- constant loads ----
            wvi = wts.tile([C, C], BF16)
            wir = wts.tile([C, C], BF16)
            eye = wts.tile([C, C], BF16)
            bmat = wts.tile([128, 14, 128], BF16)
            nc.gpsimd.dma_start(wvi[:], d_wvi[:])
            nc.gpsimd.dma_start(wir[:], d_wir[:])
            nc.gpsimd.dma_start(eye[:], d_eye[:])
            nc.gpsimd.dma_start(bmat[:], d_bmat.rearrange("m r c -> r m c"))
            l1v = wts.tile([C, C // 2], F32)
            l1i = wts.tile([C, C // 2], F32)
            b1 = wts.tile([C // 2, 1], F32)
            l2 = wts.tile([C // 2, C], F32)
            b2 = wts.tile([C, 1], F32)
            sc = wts.tile([C, 1], F32)
            bc = wts.tile([C, 1], F32)
            c1a = wts.tile([C, 8], F32)
            c1m = wts.tile([C, 8], F32)
            c2r = wts.tile([8, C], F32)
            for t, d in ((l1v, d_l1v), (l1i, d_l1i), (b1, d_b1), (l2, d_l2),
                         (b2, d_b2), (sc, d_sc), (bc, d_bc), (c1a, d_c1a),
                         (c1m, d_c1m), (c2r, d_c2r)):
                nc.gpsimd.dma_start(t[:], d[:])

            for s in range(SPC):
                # ---- loads (bf16 in HBM, no cast) ----
                fvq = [io.tile([C, QP], BF16, tag="fv", name=f"fv{s}_{k}")
                       for k in range(4)]
                fiq = [io.tile([C, QP], BF16, tag="fi", name=f"fi{s}_{k}")
                      for k in range(4)]
                for k in range(4):
                    sl = slice(k * QP, (k + 1) * QP)
                    nc.gpsimd.dma_start(fvq[k][:], d_fvi[s][:, sl])
                    nc.gpsimd.dma_start(fiq[k][:], d_fir[s][:, sl])

                def FV(sl):
                    k = sl.start // QP
                    return fvq[k][:, sl.start - k * QP:sl.stop - k * QP]

                def FI(sl):
                    k = sl.start // QP
                    return fiq[k][:, sl.start - k * QP:sl.stop - k * QP]

                # ---- pooled channel sums (DVE pairwise-add trees, bf16 2x) ----
                sv = sm.tile([C, 1], F32, tag="sv", name=f"sv{s}")
                si = sm.tile([C, 1], F32, tag="si", name=f"si{s}")
                for qt, out_col, nm in ((fvq, sv, "av"), (fiq, si, "ai")):
                    t01 = scr.tile([C, QP], BF16, tag=f"t01{nm}", name=f"t01{nm}_{s}")
                    nc.vector.tensor_tensor(out=t01[:], in0=qt[0][:], in1=qt[1][:], op=AL.add)
                    nc.vector.tensor_tensor(out=t01[:, 0:2048], in0=t01[:, 0:2048],
                                            in1=t01[:, 2048:QP], op=AL.add)
                    nc.vector.tensor_tensor(out=t01[:, 0:2048], in0=t01[:, 0:2048],
                                            in1=qt[2][:, 0:2048], op=AL.add)
                    nc.vector.tensor_tensor(out=t01[:, 0:2048], in0=t01[:, 0:2048],
                                            in1=qt[2][:, 2048:QP], op=AL.add)
                    nc.vector.tensor_tensor(out=t01[:, 0:2048], in0=t01[:, 0:2048],
                                            in1=qt[3][:, 0:2048], op=AL.add)
                    nc.vector.tensor_tensor(out=t01[:, 0:2048], in0=t01[:, 0:2048],
                                            in1=qt[3][:, 2048:QP], op=AL.add)
                    w_ = 1024
                    while w_ >= 512:
                        nc.vector.tensor_tensor(out=t01[:, 0:w_], in0=t01[:, 0:w_],
                                                in1=t01[:, w_:2 * w_], op=AL.add)
                        w_ //= 2
                    nc.vector.reduce_sum(out_col[:], t01[:, 0:512], axis=AX.X)

                # ---- channel-avg-attention MLP -> a ----
                ps1 = ps.tile([C, 1024], F32, tag="ps", name=f"ps1_{s}")
                nc.tensor.matmul(ps1[0:64, 0:1], l1v[:], sv[:], start=True, stop=False)
                nc.tensor.matmul(ps1[0:64, 0:1], l1i[:], si[:], start=False, stop=True)
                h1 = sm.tile([C // 2, 1], F32, tag="h1", name=f"h1_{s}")
                nc.scalar.activation(h1[:], ps1[0:64, 0:1], AF.Relu, bias=b1[:])
                ps2 = ps.tile([C, 1024], F32, tag="ps", name=f"ps2_{s}")
                nc.tensor.matmul(ps2[0:C, 0:1], l2[:], h1[:], start=True, stop=True)
                a_col = sm.tile([C, 1], F32, tag="a_col", name=f"a_col{s}")
                nc.scalar.activation(a_col[:], ps2[0:C, 0:1], AF.Sigmoid, bias=b2[:])
                oma = sm.tile([C, 1], F32, tag="oma", name=f"oma{s}")
                nc.vector.tensor_scalar(oma[:], a_col[:], -1.0, 1.0, AL.mult, AL.add)

                # per-sample effective conv1 weights + diag(a), diag(1-a)
                lv = sm.tile([C, C], BF16, tag="lv", name=f"lv{s}")
                li = sm.tile([C, C], BF16, tag="li", name=f"li{s}")
                dga = sm.tile([C, C], BF16, tag="dga", name=f"dga{s}")
                dgo = sm.tile([C, C], BF16, tag="dgo", name=f"dgo{s}")
                nc.vector.scalar_tensor_tensor(lv[:], wir[:], a_col[:], wvi[:], AL.mult, AL.add)
                nc.vector.scalar_tensor_tensor(li[:], wvi[:], a_col[:], wir[:], AL.mult, AL.add)
                nc.vector.tensor_scalar(dga[:], eye[:], a_col[:], 0.0, AL.mult, AL.add)
                nc.vector.tensor_scalar(dgo[:], eye[:], oma[:], 0.0, AL.mult, AL.add)

                # ---- conv1 + per-quarter maps ----
                avp = sm.tile([C, 16], F32, tag="avp", name=f"avp{s}")
                rmax = sm.tile([C, 128], BF16, tag="rmax", name=f"rmax{s}")
                maxpad = mp.tile([128, 134], BF16, tag="maxpad", name=f"maxpad{s}")
                sumpad = mp.tile([128, 134], BF16, tag="sumpad", name=f"sumpad{s}")
                nc.vector.memset(maxpad[:, 0:3], 0.0)
                nc.vector.memset(maxpad[:, 131:134], 0.0)
                nc.vector.memset(sumpad[:, 0:3], 0.0)
                nc.vector.memset(sumpad[:, 131:134], 0.0)

                for k in range(4):
                    gq = gqp.tile([C, QP], BF16, tag="gq", name=f"gq{s}_{k}")
                    for j in range(4):
                        pg = ps.tile([C, 1024], F32, tag="ps", name=f"pg{s}_{k}_{j}")
                        base = k * QP + j * 1024
                        # group matmuls by stationary operand (lv then li)
                        nc.tensor.matmul(pg[:, 0:512], lv[:], FV(slice(base, base + 512)),
                                         start=True, stop=False)
                        nc.tensor.matmul(pg[:, 512:1024], lv[:], FV(slice(base + 512, base + 1024)),
                                         start=True, stop=False)
                        nc.tensor.matmul(pg[:, 0:512], li[:], FI(slice(base, base + 512)),
                                         start=False, stop=True)
                        nc.tensor.matmul(pg[:, 512:1024], li[:], FI(slice(base + 512, base + 1024)),
                                         start=False, stop=True)
                        nc.scalar.activation(gq[:, j * 1024:(j + 1) * 1024], pg[:],
                                             AF.Relu, bias=bc[:], scale=sc[:],
                                             accum_out=avp[:, 4 * k + j:4 * k + j + 1])
                    # transpose quarter -> (w, h, c)
                    gt = gtb.tile([128, QP], BF16, tag="gt", name=f"gt{s}_{k}")
                    gt3 = gt[:].rearrange("p (h c) -> p h c", c=128)
                    nc.sync.dma_start_transpose(gt3, gq[:])
                    # channel-mean map: segmented reduce over c into a
                    # contiguous fp32 tile, then a tiny copy into the map
                    msum = sm.tile([128, 32], F32, tag="msum", name=f"msum{s}_{k}")
                    nc.vector.reduce_sum(
                        msum[:].rearrange("p (f o) -> p f o", o=1), gt3, axis=AX.X)
                    nc.vector.tensor_copy(
                        sumpad[:, 3 + k * 32:3 + k * 32 + 32], msum[:])
                    # channel-max tree (in place on gt)
                    w_ = 64
                    while w_ >= 1:
                        nc.vector.tensor_tensor(
                            out=gt3[:, :, 0:w_], in0=gt3[:, :, 0:w_],
                            in1=gt3[:, :, w_:2 * w_], op=AL.max)
                        w_ //= 2
                    nc.vector.tensor_copy(
                        maxpad[:, 3 + k * 32:3 + k * 32 + 32].rearrange("p (f o) -> p f o", o=1),
                        gt3[:, :, 0:1])
                    # pixel-max tree in place on gq (dead after the transpose)
                    w_ = 2048
                    while w_ >= 128:
                        nc.vector.tensor_tensor(out=gq[:, 0:w_], in0=gq[:, 0:w_],
                                                in1=gq[:, w_:2 * w_], op=AL.max)
                        w_ //= 2
                    if k == 0:
                        nc.vector.tensor_copy(rmax[:], gq[:, 0:128])
                    else:
                        nc.vector.tensor_tensor(out=rmax[:], in0=rmax[:],
                                                in1=gq[:, 0:128], op=AL.max)

                avs = sm.tile([C, 1], F32, tag="avs", name=f"avs{s}")
                nc.vector.reduce_sum(avs[:], avp[:], axis=AX.X)
                mx = sm.tile([C, 1], F32, tag="mx", name=f"mx{s}")
                nc.vector.reduce_max(mx[:], rmax[:], axis=AX.X)

                # ---- ChannelAttention MLP -> ca column ----
                psa = ps.tile([C, 1024], F32, tag="ps", name=f"psa{s}")
                nc.tensor.matmul(psa[0:8, 0:1], c1a[:], avs[:], start=True, stop=True)
                ha = sm.tile([8, 1], F32, tag="ha", name=f"ha{s}")
                nc.scalar.activation(ha[:], psa[0:8, 0:1], AF.Relu)
                psm = ps.tile([C, 1024], F32, tag="ps", name=f"psm{s}")
                nc.tensor.matmul(psm[0:8, 0:1], c1m[:], mx[:], start=True, stop=True)
                hm = sm.tile([8, 1], F32, tag="hm", name=f"hm{s}")
                nc.scalar.activation(hm[:], psm[0:8, 0:1], AF.Relu)
                psr = ps.tile([C, 1024], F32, tag="ps", name=f"psr{s}")
                nc.tensor.matmul(psr[0:1, 0:C], ha[:], c2r[:], start=True, stop=False)
                nc.tensor.matmul(psr[0:1, 0:C], hm[:], c2r[:], start=False, stop=True)
                ca = sm.tile([1, C], BF16, tag="ca", name=f"ca{s}")
                nc.scalar.activation(ca[:], psr[0:1, 0:C], AF.Sigmoid)

                # ---- SpatialAttention: 7x7 conv as banded matmuls ----
                pss = ps.tile([C, 1024], F32, tag="ps", name=f"pss{s}")
                first = True
                for chn, pad in ((0, sumpad), (1, maxpad)):
                    for dy in range(7):
                        nc.tensor.matmul(pss[0:128, 0:128], pad[:, dy:dy + 128],
                                         bmat[:, chn * 7 + dy, :],
                                         start=first, stop=(chn == 1 and dy == 6))
                        first = False
                sa_hw = sm.tile([128, 128], BF16, tag="sa_hw", name=f"sa_hw{s}")
                nc.scalar.activation(sa_hw[:], pss[0:128, 0:128], AF.Sigmoid)


                # ---- blend: 8 blocks of 2048 px ----
                for b in range(8):
                    b0 = b * 2048
                    # stage block's 16 sa rows on partition 0; tt = sigmoid(ca x sa)
                    # via K=1 matmuls (rank-1 broadcast on the PE)
                    sa1p = sfl.tile([1, 2048], BF16, tag="sa1p", name=f"sa1p{s}_{b}")
                    nc.scalar.dma_start(sa1p[:], sa_hw[16 * b:16 * b + 16, :])
                    tt = bl.tile([C, 2048], BF16, tag="tt", name=f"tt{s}_{b}")
                    for j in range(2):
                        ppw = ps.tile([C, 1024], F32, tag="ps", name=f"ppw{s}_{b}_{j}")
                        nc.tensor.matmul(ppw[:, 0:512], ca[:],
                                         sa1p[0:1, j * 1024:j * 1024 + 512],
                                         start=True, stop=True)
                        nc.tensor.matmul(ppw[:, 512:1024], ca[:],
                                         sa1p[0:1, j * 1024 + 512:(j + 1) * 1024],
                                         start=True, stop=True)
                        nc.scalar.activation(tt[:, j * 1024:(j + 1) * 1024], ppw[:],
                                             AF.Sigmoid)
                    eb = bl.tile([C, 2048], BF16, tag="eb", name=f"eb{s}_{b}")
                    nc.vector.tensor_tensor(out=eb[:], in0=FV(slice(b0, b0 + 2048)),
                                            in1=FI(slice(b0, b0 + 2048)), op=AL.subtract)
                    dtb = bl.tile([C, 2048], BF16, tag="dtb", name=f"dtb{s}_{b}")
                    nc.vector.tensor_tensor(out=dtb[:], in0=tt[:], in1=eb[:], op=AL.mult)
                    for j in range(2):
                        base = b0 + j * 1024
                        act_mode = (2 * b + j) % (16 // max(1, ACT_EVAC)) == 0 if ACT_EVAC else False
                        pb = ps.tile([C, 1024], F32, tag="ps", name=f"pb{s}_{b}_{j}")
                        sla = slice(base, base + 512)
                        slb = slice(base + 512, base + 1024)
                        nc.tensor.matmul(pb[:, 0:512], dga[:], FV(sla), start=True, stop=False)
                        nc.tensor.matmul(pb[:, 512:1024], dga[:], FV(slb), start=True, stop=False)
                        if act_mode:
                            nc.tensor.matmul(pb[:, 0:512], eye[:], FI(sla), start=False, stop=False)
                            nc.tensor.matmul(pb[:, 512:1024], eye[:], FI(slb), start=False, stop=False)
                            nc.tensor.matmul(pb[:, 0:512], dgo[:],
                                             dtb[:, j * 1024:j * 1024 + 512],
                                             start=False, stop=True)
                            nc.tensor.matmul(pb[:, 512:1024], dgo[:],
                                             dtb[:, j * 1024 + 512:(j + 1) * 1024],
                                             start=False, stop=True)
                        else:
                            nc.tensor.matmul(pb[:, 0:512], eye[:], FI(sla), start=False, stop=True)
                            nc.tensor.matmul(pb[:, 512:1024], eye[:], FI(slb), start=False, stop=True)
                        ob = obp.tile([C, 1024], BF16, tag="ob", name=f"ob{s}_{b}_{j}")
                        if act_mode:
                            nc.scalar.activation(ob[:], pb[:], AF.Copy)
                        else:
                            # out = pb + (1-a)*dt, fused into the PSUM evac
                            nc.vector.scalar_tensor_tensor(
                                ob[:], dtb[:, j * 1024:(j + 1) * 1024], oma[:], pb[:],
                                AL.mult, AL.add)
                        nc.sync.dma_start(d_out[s][:, base:base + 1024], ob[:])

    nc.compile()
    return nc


def _host_consts(ca1_w, ca1_b, bn_a_g, bn_a_b, bn_a_m, bn_a_v,
                 ca2_w, ca2_b, bn_b_g, bn_b_b, bn_b_m, bn_b_v,
                 conv1_w, conv1_b, bn_c_g, bn_c_b, bn_c_m, bn_c_v,
                 chatt_w1, chatt_w2, sa_w):
    bf = ml_dtypes.bfloat16
    f = np.float32
    k_a = bn_a_g / np.sqrt(bn_a_v + EPS)
    w1 = ca1_w * k_a[:, None]
    b1 = (ca1_b - bn_a_m) * k_a + bn_a_b
    k_b = bn_b_g / np.sqrt(bn_b_v + EPS)
    w2 = ca2_w * k_b[:, None]
    b2 = (ca2_b - bn_b_m) * k_b + bn_b_b
    s_c = bn_c_g / np.sqrt(bn_c_v + EPS)
    b_c = (conv1_b - bn_c_m) * s_c + bn_c_b
    bmat = np.zeros((14, 128, 128), np.float32)
    for chn in range(2):
        scale = (1.0 / 128.0) if chn == 0 else 1.0
        for dy in range(7):
            for dx in range(7):
                off = dx - 3          # w' - w
                v = sa_w[0, chn, dy, dx] * scale
                if off >= 0:
                    idx = np.arange(0, 128 - off)
                    bmat[chn * 7 + dy, idx + off, idx] = v
                else:
                    idx = np.arange(-off, 128)
                    bmat[chn * 7 + dy, idx + off, idx] = v
    return {
        "wvi_t": np.ascontiguousarray(conv1_w[:, :C].T).astype(bf),
        "wir_t": np.ascontiguousarray(conv1_w[:, C:].T).astype(bf),
        "eye": np.eye(C, dtype=f).astype(bf),
        "bmat": bmat.astype(bf),
        "l1v": np.ascontiguousarray((w1[:, :C] / P).T).astype(f),
        "l1i": np.ascontiguousarray((w1[:, C:] / P).T).astype(f),
        "b1": b1.reshape(-1, 1).astype(f),
        "l2": np.ascontiguousarray(w2.T).astype(f),
        "b2": b2.reshape(-1, 1).astype(f),
        "sc": s_c.reshape(-1, 1).astype(f),
        "bc": b_c.reshape(-1, 1).astype(f),
        "c1a": np.ascontiguousarray((chatt_w1 / P).T).astype(f),
        "c1m": np.ascontiguousarray(chatt_w1.T).astype(f),
        "c2r": np.ascontiguousarray(chatt_w2.T).astype(f),
    }


def kernel(f_vi, f_ir, ca1_w, ca1_b, bn_a_g, bn_a_b, bn_a_m, bn_a_v,
           ca2_w, ca2_b, bn_b_g, bn_b_b, bn_b_m, bn_b_v,
           conv1_w, conv1_b, bn_c_g, bn_c_b, bn_c_m, bn_c_v,
           chatt_w1, chatt_w2, sa_w, _trace=False):
    if "nc" not in _cache:
        _cache["nc"] = _build_program()
    nc = _cache["nc"]

    consts = _host_consts(
        np.asarray(ca1_w, np.float32), np.asarray(ca1_b, np.float32),
        np.asarray(bn_a_g, np.float32), np.asarray(bn_a_b, np.float32),
        np.asarray(bn_a_m, np.float32), np.asarray(bn_a_v, np.float32),
        np.asarray(ca2_w, np.float32), np.asarray(ca2_b, np.float32),
        np.asarray(bn_b_g, np.float32), np.asarray(bn_b_b, np.float32),
        np.asarray(bn_b_m, np.float32), np.asarray(bn_b_v, np.float32),
        np.asarray(conv1_w, np.float32), np.asarray(conv1_b, np.float32),
        np.asarray(bn_c_g, np.float32), np.asarray(bn_c_b, np.float32),
        np.asarray(bn_c_m, np.float32), np.asarray(bn_c_v, np.float32),
        np.asarray(chatt_w1, np.float32), np.asarray(chatt_w2, np.float32),
        np.asarray(sa_w, np.float32))

    bf = ml_dtypes.bfloat16
    fv = np.asarray(f_vi, np.float32).reshape(N, C, P).astype(bf)
    fi = np.asarray(f_ir, np.float32).reshape(N, C, P).astype(bf)
    in_maps = []
    for i in range(NCORES):
        m = dict(consts)
        m["f_vi"] = np.ascontiguousarray(fv[i * SPC:(i + 1) * SPC])
        m["f_ir"] = np.ascontiguousarray(fi[i * SPC:(i + 1) * SPC])
        in_maps.append(m)

    res = bass_utils.run_bass_kernel_spmd(nc, in_maps, core_ids=list(range(NCORES)),
                                          trace=_trace)
    if _trace:
        _cache["last_trace"] = res
    out = np.concatenate([res.results[i]["out"] for i in range(NCORES)], axis=0)
    return out.astype(np.float32).reshape(N, C, H, W)
